# revision 7
# baseline (speedup 1.0000x reference)
"""Bass/Tile TRN2 kernel for nn_AttentionHead: single-head attention with
q/k/v projections (512->64), key mask, softmax over 4096 keys.

Sharding: 8 cores; core c handles batch c//2, query-half c%2 (2048 queries),
with that batch's full k/v replicated. No collectives.

Per-core dataflow:
  - SWDGE cast-DMA loads q/k/v fp32 -> bf16 staged [t,d] tiles
  - PE transposes 128x128 blocks -> qT/kT/vT in [d, t] layout
  - projections on TensorE: QT/KT = W.T @ xT  ([e, t] layout), V natural [t2, e]
    with a constant ones column appended -> V1 [t2, 65]
  - scores: S^T chunks [t2=128, t1] = KT_chunk.T @ QT  (contract e=64)
  - ScalarE: exp(0.125 * S^T + maskbias) fused (maskbias = (mask-1)*1e9 per key)
  - PV: O^T[65, t1] += V1_chunk.T @ expS chunk  (row 64 = softmax denominator)
  - epilogue: PE transpose [65,128] blocks, reciprocal + scale on VectorE
"""

import sys
import types

import numpy as np

import concourse.bass as bass
import concourse.tile as tile
from concourse import bacc, mybir
from concourse.masks import make_identity

B, T1, T2, D, E = 4, 4096, 4096, 512, 64
P = 128
F32 = mybir.dt.float32
BF16 = mybir.dt.bfloat16
EXPF = mybir.ActivationFunctionType.Exp
MULT = mybir.AluOpType.mult
ADD = mybir.AluOpType.add


def _install_ntff_hook():
    """Make trace=True usable under axon when antenv.axon_hooks is absent."""
    try:
        import antenv.axon_hooks  # noqa: F401
        return
    except ImportError:
        pass
    try:
        from trn_agent_boot.trn_boot import _ntff_profile_via_ctypes
        hook = _ntff_profile_via_ctypes("/opt/axon/libaxon_pjrt.so")
    except Exception:
        hook = None
    mod = types.ModuleType("antenv.axon_hooks")
    mod.get_axon_ntff_profile_hook = lambda: hook
    mod.set_axon_ntff_profile_hook = lambda h: None
    sys.modules["antenv.axon_hooks"] = mod


def _bcast_ap(ap, parts):
    """Broadcast a 1-D DRAM AP across `parts` partitions (stride-0 DMA)."""
    return bass.AP(tensor=ap.tensor, offset=ap.offset, ap=[[0, parts], ap.ap[0]])


def build_body(tc, nc, q, k, v, mask, Wq, bq, Wk, bk, Wv, bv, out, t1l, t2):
    DC = D // P            # 4 d-chunks
    NT2 = t2 // P          # t2 chunks of 128
    NT1 = t1l // P
    TB = 512               # staging/projection block (t rows)
    T1B = min(1024, t1l)   # phase-B t1 pass width

    with (
        tc.tile_pool(name="consts", bufs=1) as consts,
        tc.tile_pool(name="persist", bufs=1) as persist,
    ):
        ident_b = consts.tile([P, P], BF16)
        make_identity(nc, ident_b)
        ident_f = consts.tile([P, P], F32)
        make_identity(nc, ident_f)

        # weights, bf16, d on partitions: [P, DC, E]
        wq_b = consts.tile([P, DC, E], BF16)
        nc.gpsimd.dma_start(out=wq_b, in_=Wq.rearrange("(c p) e -> p c e", p=P))
        wk_b = consts.tile([P, DC, E], BF16)
        nc.gpsimd.dma_start(out=wk_b, in_=Wk.rearrange("(c p) e -> p c e", p=P))
        wv_b = consts.tile([P, DC, E], BF16)
        nc.gpsimd.dma_start(out=wv_b, in_=Wv.rearrange("(c p) e -> p c e", p=P))

        # biases: per-partition [E, 1] for QT/KT evac; broadcast [P, E] for V
        bq_s = consts.tile([E, 1], F32)
        nc.sync.dma_start(out=bq_s, in_=bq[:, None])
        bk_s = consts.tile([E, 1], F32)
        nc.sync.dma_start(out=bk_s, in_=bk[:, None])
        bv_bc = consts.tile([P, E], F32)
        nc.gpsimd.dma_start(out=bv_bc, in_=_bcast_ap(bv, P))

        # mask -> additive bias per key: 0 where mask=1, -1e9 where mask=0
        mk = consts.tile([P, NT2], F32)
        nc.sync.dma_start(out=mk, in_=mask.rearrange("(c p) -> p c", p=P))
        maskbias = consts.tile([P, NT2], F32)
        nc.vector.tensor_scalar(maskbias, mk, 1e9, -1e9, MULT, ADD)

        qT = persist.tile([P, DC, t1l], BF16)
        kT = persist.tile([P, DC, t2], BF16)
        vT = persist.tile([P, DC, t2], BF16)
        QT = persist.tile([E, t1l], BF16)
        KT = persist.tile([E, t2], BF16)
        V1 = persist.tile([P, NT2, E + 1], BF16)
        out_sb = persist.tile([P, NT1, E], F32)

        nc.vector.memset(V1, 1.0)  # ones column survives in col E

        # ---------------- phase A: load, transpose, project ----------------
        with (
            tc.tile_pool(name="stage", bufs=3) as stagep,
            tc.tile_pool(name="psA", bufs=2, space="PSUM") as psA,
        ):
            def load_transpose(src, dst_T, nrows):
                srcr = src.rearrange("(n p) d -> p n d", p=P)
                tb_sz = min(TB, nrows)
                nsub = tb_sz // P
                for tb in range(nrows // tb_sz):
                    st = stagep.tile([P, nsub, D], BF16, tag="stage")
                    nc.gpsimd.dma_start(
                        out=st, in_=srcr[:, tb * nsub:(tb + 1) * nsub, :])
                    for ns in range(nsub):
                        pst = psA.tile([P, D], BF16, tag="tps")
                        for j in range(DC):
                            nc.tensor.transpose(
                                pst[:, j * P:(j + 1) * P],
                                st[:, ns, j * P:(j + 1) * P], ident_b)
                        t0 = tb * tb_sz + ns * P
                        nc.vector.tensor_copy(
                            out=dst_T[:, :, t0:t0 + P],
                            in_=pst.rearrange("p (j c) -> p j c", c=P))

            def project_T(src_T, w_b, b_s, dst, nrows):
                tb_sz = min(TB, nrows)
                for tb in range(nrows // tb_sz):
                    ps = psA.tile([E, tb_sz], F32, tag="pproj")
                    for j in range(DC):
                        nc.tensor.matmul(
                            ps, w_b[:, j],
                            src_T[:, j, tb * tb_sz:(tb + 1) * tb_sz],
                            start=(j == 0), stop=(j == DC - 1))
                    nc.vector.tensor_scalar_add(
                        dst[:, tb * tb_sz:(tb + 1) * tb_sz], ps, b_s)

            load_transpose(q, qT, t1l)
            project_T(qT, wq_b, bq_s, QT, t1l)
            load_transpose(k, kT, t2)
            project_T(kT, wk_b, bk_s, KT, t2)
            load_transpose(v, vT, t2)
            for c in range(NT2):
                ps = psA.tile([P, E], F32, tag="pvp")
                for j in range(DC):
                    nc.tensor.matmul(
                        ps, vT[:, j, c * P:(c + 1) * P], wv_b[:, j],
                        start=(j == 0), stop=(j == DC - 1))
                nc.vector.tensor_add(V1[:, c, 0:E], ps, bv_bc)

        # ---------------- phase B: scores -> exp -> PV ----------------
        with (
            tc.tile_pool(name="psS", bufs=2, space="PSUM") as psS,
            tc.tile_pool(name="psPV", bufs=1, space="PSUM") as psPV,
            tc.tile_pool(name="psO", bufs=2, space="PSUM") as psO,
            tc.tile_pool(name="expp", bufs=3) as expp,
            tc.tile_pool(name="ep", bufs=2) as ep,
        ):
            HW = min(512, T1B)  # matmul half width (psum bank = 512 f32)
            NH = T1B // HW
            for pi in range(t1l // T1B):
                pv = [psPV.tile([E + 1, HW], F32, tag=f"pv{h}",
                                name=f"pv_{pi}_{h}")
                      for h in range(NH)]
                for c in range(NT2):
                    ps = psS.tile([P, T1B], F32, tag="s")
                    for h in range(NH):
                        nc.tensor.matmul(
                            ps[:, h * HW:(h + 1) * HW],
                            KT[:, c * P:(c + 1) * P],
                            QT[:, pi * T1B + h * HW: pi * T1B + (h + 1) * HW],
                            start=True, stop=True)
                    ex = expp.tile([P, T1B], BF16, tag="e")
                    nc.scalar.activation(out=ex, in_=ps, func=EXPF,
                                         bias=maskbias[:, c:c + 1], scale=0.125)
                    for h in range(NH):
                        nc.tensor.matmul(
                            pv[h], V1[:, c, :], ex[:, h * HW:(h + 1) * HW],
                            start=(c == 0), stop=(c == NT2 - 1))
                for h in range(NH):
                    ov = ep.tile([E + 1, HW], F32, tag="ov")
                    nc.vector.tensor_copy(out=ov, in_=pv[h])
                    for j in range(HW // P):
                        po = psO.tile([P, E + 1], F32, tag="o")
                        nc.tensor.transpose(
                            po, ov[:, j * P:(j + 1) * P],
                            ident_f[0:E + 1, 0:E + 1])
                        rec = ep.tile([P, 1], F32, tag="rec")
                        nc.vector.reciprocal(rec, po[:, E:E + 1])
                        n_idx = (pi * T1B + h * HW + j * P) // P
                        nc.vector.tensor_scalar_mul(
                            out_sb[:, n_idx, :], po[:, 0:E], rec)

        nc.sync.dma_start(out=out.rearrange("(n p) e -> p n e", p=P),
                          in_=out_sb)


def build_nc(t1l=T1 // 2, t2=T2):
    nc = bacc.Bacc()
    q = nc.declare_dram_parameter("q", [t1l, D], F32, isOutput=False)
    k = nc.declare_dram_parameter("k", [t2, D], F32, isOutput=False)
    v = nc.declare_dram_parameter("v", [t2, D], F32, isOutput=False)
    mask = nc.declare_dram_parameter("mask", [t2], F32, isOutput=False)
    Wq = nc.declare_dram_parameter("Wq", [D, E], F32, isOutput=False)
    bq = nc.declare_dram_parameter("bq", [E], F32, isOutput=False)
    Wk = nc.declare_dram_parameter("Wk", [D, E], F32, isOutput=False)
    bk = nc.declare_dram_parameter("bk", [E], F32, isOutput=False)
    Wv = nc.declare_dram_parameter("Wv", [D, E], F32, isOutput=False)
    bv = nc.declare_dram_parameter("bv", [E], F32, isOutput=False)
    out = nc.declare_dram_parameter("out", [t1l, E], F32, isOutput=True)
    with tile.TileContext(nc) as tc:
        build_body(tc, nc, q[:], k[:], v[:], mask[:], Wq[:], bq[:], Wk[:],
                   bk[:], Wv[:], bv[:], out[:], t1l, t2)
    nc.compile()
    return nc


_NC_CACHE = {}


def _get_nc():
    if "nc" not in _NC_CACHE:
        _NC_CACHE["nc"] = build_nc()
    return _NC_CACHE["nc"]


def make_in_maps(q, k, v, mask, Wq, bq, Wk, bk, Wv, bv):
    t1l = T1 // 2
    shared = {
        "Wq": np.ascontiguousarray(Wq, np.float32),
        "bq": np.ascontiguousarray(bq, np.float32),
        "Wk": np.ascontiguousarray(Wk, np.float32),
        "bk": np.ascontiguousarray(bk, np.float32),
        "Wv": np.ascontiguousarray(Wv, np.float32),
        "bv": np.ascontiguousarray(bv, np.float32),
    }
    in_maps = []
    for c in range(8):
        b, h = divmod(c, 2)
        in_maps.append({
            "q": np.ascontiguousarray(q[b, h * t1l:(h + 1) * t1l], np.float32),
            "k": np.ascontiguousarray(k[b], np.float32),
            "v": np.ascontiguousarray(v[b], np.float32),
            "mask": np.ascontiguousarray(mask[b, 0], np.float32),
            **shared,
        })
    return in_maps


def assemble_out(results):
    t1l = T1 // 2
    out = np.empty((B, T1, E), np.float32)
    for c in range(8):
        b, h = divmod(c, 2)
        out[b, h * t1l:(h + 1) * t1l] = results[c]["out"]
    return out


def run(inputs, trace=False):
    from concourse.bass_utils import run_bass_kernel_spmd
    _install_ntff_hook()
    nc = _get_nc()
    in_maps = make_in_maps(**inputs)
    res = run_bass_kernel_spmd(nc, in_maps, list(range(8)), trace=trace)
    return assemble_out(res.results), res


def kernel(q, k, v, mask, Wq, bq, Wk, bk, Wv, bv):
    out, _ = run(dict(q=q, k=k, v=v, mask=mask, Wq=Wq, bq=bq, Wk=Wk, bk=bk,
                      Wv=Wv, bv=bv))
    return out


# revision 11
# speedup vs baseline: 1.3797x; 1.3797x over previous
"""Bass/Tile TRN2 kernel for nn_AttentionHead: single-head attention with
q/k/v projections (512->64), key mask, softmax over 4096 keys.

Sharding: 8 cores; core c handles batch c//2, query-half c%2 (2048 queries),
with that batch's full k/v replicated. No collectives.

Per-core dataflow:
  - SWDGE cast-DMA loads q/k/v fp32 -> bf16 staged [t,d] tiles
  - PE transposes 128x128 blocks -> qT/kT/vT in [d, t] layout
  - projections on TensorE: QT/KT = W.T @ xT  ([e, t] layout), V natural [t2, e]
    with a constant ones column appended -> V1 [t2, 65]
  - scores: S^T chunks [t2=128, t1] = KT_chunk.T @ QT  (contract e=64)
  - ScalarE: exp(0.125 * S^T + maskbias) fused (maskbias = (mask-1)*1e9 per key)
  - PV: O^T[65, t1] += V1_chunk.T @ expS chunk  (row 64 = softmax denominator)
  - epilogue: PE transpose [65,128] blocks, reciprocal + scale on VectorE
"""

import sys
import types

import numpy as np

import concourse.bass as bass
import concourse.tile as tile
from concourse import bacc, mybir
from concourse.masks import make_identity

B, T1, T2, D, E = 4, 4096, 4096, 512, 64
P = 128
F32 = mybir.dt.float32
BF16 = mybir.dt.bfloat16
EXPF = mybir.ActivationFunctionType.Exp
MULT = mybir.AluOpType.mult
ADD = mybir.AluOpType.add


def _install_ntff_hook():
    """Make trace=True usable under axon when antenv.axon_hooks is absent."""
    try:
        import antenv.axon_hooks  # noqa: F401
        return
    except ImportError:
        pass
    try:
        from trn_agent_boot.trn_boot import _ntff_profile_via_ctypes
        hook = _ntff_profile_via_ctypes("/opt/axon/libaxon_pjrt.so")
    except Exception:
        hook = None
    mod = types.ModuleType("antenv.axon_hooks")
    mod.get_axon_ntff_profile_hook = lambda: hook
    mod.set_axon_ntff_profile_hook = lambda h: None
    sys.modules["antenv.axon_hooks"] = mod


def _bcast_ap(ap, parts):
    """Broadcast a 1-D DRAM AP across `parts` partitions (stride-0 DMA)."""
    return bass.AP(tensor=ap.tensor, offset=ap.offset, ap=[[0, parts], ap.ap[0]])


def build_body(tc, nc, q, k, v, mask, Wq, bq, Wk, bk, Wv, bv, out, t1l, t2):
    DC = D // P            # 4 d-chunks
    NT2 = t2 // P          # t2 chunks of 128
    NT1 = t1l // P
    TB = 512               # staging/projection block (t rows)
    T1B = min(1024, t1l)   # phase-B t1 pass width

    with (
        tc.tile_pool(name="consts", bufs=1) as consts,
        tc.tile_pool(name="persist", bufs=1) as persist,
    ):
        ident_b = consts.tile([P, P], BF16)
        make_identity(nc, ident_b)
        ident_f = consts.tile([P, P], F32)
        make_identity(nc, ident_f)

        # weights, bf16, d on partitions: [P, DC, E]
        wq_b = consts.tile([P, DC, E], BF16)
        nc.gpsimd.dma_start(out=wq_b, in_=Wq.rearrange("(c p) e -> p c e", p=P))
        wk_b = consts.tile([P, DC, E], BF16)
        nc.gpsimd.dma_start(out=wk_b, in_=Wk.rearrange("(c p) e -> p c e", p=P))
        wv_b = consts.tile([P, DC, E], BF16)
        nc.gpsimd.dma_start(out=wv_b, in_=Wv.rearrange("(c p) e -> p c e", p=P))

        # biases: per-partition [E, 1] for QT/KT evac; broadcast [P, E] for V
        bq_s = consts.tile([E, 1], F32)
        nc.sync.dma_start(out=bq_s, in_=bq[:, None])
        bk_s = consts.tile([E, 1], F32)
        nc.sync.dma_start(out=bk_s, in_=bk[:, None])
        bv_bc = consts.tile([P, E], F32)
        nc.gpsimd.dma_start(out=bv_bc, in_=_bcast_ap(bv, P))

        # mask values per key, [partition = t2 % 128, col = t2 // 128]
        mk = consts.tile([P, NT2], F32)
        nc.sync.dma_start(out=mk, in_=mask.rearrange("(c p) -> p c", p=P))

        qT = persist.tile([P, DC, t1l], BF16)
        kT = persist.tile([P, DC, t2], BF16)
        vT = persist.tile([P, DC, t2], BF16)
        # QT/KT duplicated on partitions 64-127 for row-packed score matmuls
        QT = persist.tile([P, t1l], BF16)
        KT = persist.tile([P, t2], BF16)
        V1 = persist.tile([P, NT2, E + 1], BF16)
        out_sb = persist.tile([P, NT1, E], F32)

        nc.vector.memset(V1, 1.0)  # ones column survives in col E

        # ---------------- phase A: load, transpose, project ----------------
        with (
            tc.tile_pool(name="stage", bufs=3) as stagep,
            tc.tile_pool(name="psA", bufs=2, space="PSUM") as psA,
        ):
            evac_flip = [0]

            def evac_copy(out_ap, in_ap):
                # alternate DVE / ACT so neither engine is the evac bottleneck
                evac_flip[0] ^= 1
                if evac_flip[0]:
                    nc.vector.tensor_copy(out=out_ap, in_=in_ap)
                else:
                    nc.scalar.copy(out=out_ap, in_=in_ap)

            def load_transpose(src, dst_T, nrows):
                srcr = src.rearrange("(n p) d -> p n d", p=P)
                tb_sz = min(TB, nrows)
                nsub = tb_sz // P
                for tb in range(nrows // tb_sz):
                    st = stagep.tile([P, nsub, D], BF16, tag="stage")
                    nc.gpsimd.dma_start(
                        out=st, in_=srcr[:, tb * nsub:(tb + 1) * nsub, :])
                    for ns in range(nsub):
                        pst = psA.tile([P, D], BF16, tag="tps")
                        for j in range(DC):
                            nc.tensor.transpose(
                                pst[:, j * P:(j + 1) * P],
                                st[:, ns, j * P:(j + 1) * P], ident_b)
                        t0 = tb * tb_sz + ns * P
                        evac_copy(dst_T[:, :, t0:t0 + P],
                                  pst.rearrange("p (j c) -> p j c", c=P))

            def project_T(src_T, w_b, b_s, dst, nrows):
                # dst is [128, nrows]; write the projection to partitions 0:64
                # and a duplicate to 64:128 (for row-packed score matmuls)
                tb_sz = min(TB, nrows)
                for tb in range(nrows // tb_sz):
                    ps = psA.tile([E, tb_sz], F32, tag="pproj")
                    for j in range(DC):
                        nc.tensor.matmul(
                            ps, w_b[:, j],
                            src_T[:, j, tb * tb_sz:(tb + 1) * tb_sz],
                            start=(j == 0), stop=(j == DC - 1))
                    sl = slice(tb * tb_sz, (tb + 1) * tb_sz)
                    nc.vector.tensor_scalar_add(dst[0:E, sl], ps, b_s)
                    nc.scalar.activation(
                        out=dst[E:2 * E, sl], in_=ps,
                        func=mybir.ActivationFunctionType.Identity,
                        bias=b_s, scale=1.0)

            load_transpose(q, qT, t1l)
            project_T(qT, wq_b, bq_s, QT, t1l)
            load_transpose(k, kT, t2)
            project_T(kT, wk_b, bk_s, KT, t2)
            load_transpose(v, vT, t2)
            for c in range(NT2):
                ps = psA.tile([P, E], F32, tag="pvp")
                for j in range(DC):
                    nc.tensor.matmul(
                        ps, vT[:, j, c * P:(c + 1) * P], wv_b[:, j],
                        start=(j == 0), stop=(j == DC - 1))
                nc.vector.tensor_add(V1[:, c, 0:E], ps, bv_bc)
                # fold the key mask into V rows AND the ones column: the
                # masked softmax is exactly sum(mask*exp) / uses mask*V rows
                nc.vector.tensor_scalar_mul(V1[:, c, :], V1[:, c, :],
                                            mk[:, c:c + 1])

        # ---------------- phase B: scores -> exp -> PV ----------------
        # per t1-half of 512: stream key-chunk PAIRS. The two score matmuls
        # of a pair run CONCURRENTLY in array rows 0-63 / 64-127
        # (tile_position row packing, contract dim is only 64), write the two
        # halves of one [128, 1024] psum tile, and share one exp call.
        with (
            tc.tile_pool(name="psS", bufs=3, space="PSUM") as psS,
            tc.tile_pool(name="psPV", bufs=1, space="PSUM") as psPV,
            tc.tile_pool(name="psO", bufs=1, space="PSUM") as psO,
            tc.tile_pool(name="expp", bufs=3) as expp,
            tc.tile_pool(name="ep", bufs=2) as ep,
        ):
            HW = min(512, T1B)
            NP2 = max(1, NT2 // 2)  # chunk pairs
            for hi in range(t1l // HW):
                q0 = hi * HW
                pvt = psPV.tile([E + 1, HW], F32, tag="pv",
                                name=f"pv_{hi}")
                for pc in range(NP2):
                    c0, c1 = 2 * pc, 2 * pc + 1
                    ps = psS.tile([P, 2 * HW], F32, tag="s")
                    nc.tensor.matmul(
                        ps[:, 0:HW], KT[0:E, c0 * P:(c0 + 1) * P],
                        QT[0:E, q0:q0 + HW], start=True, stop=True,
                        tile_position=(0, 0))
                    nc.tensor.matmul(
                        ps[:, HW:2 * HW], KT[E:2 * E, c1 * P:(c1 + 1) * P],
                        QT[E:2 * E, q0:q0 + HW], start=True, stop=True,
                        tile_position=(64, 0))
                    ex = expp.tile([P, 2 * HW], BF16, tag="e")
                    nc.scalar.activation(out=ex, in_=ps, func=EXPF,
                                         scale=0.125)
                    nc.tensor.matmul(pvt, V1[:, c0, :], ex[:, 0:HW],
                                     start=(pc == 0), stop=False)
                    nc.tensor.matmul(pvt, V1[:, c1, :], ex[:, HW:2 * HW],
                                     start=False, stop=(pc == NP2 - 1))
                ov = ep.tile([E + 1, HW], F32, tag="ov")
                nc.vector.tensor_copy(out=ov, in_=pvt)
                for j in range(HW // P):
                    po = psO.tile([P, E + 1], F32, tag="o")
                    nc.tensor.transpose(
                        po, ov[:, j * P:(j + 1) * P],
                        ident_f[0:E + 1, 0:E + 1])
                    rec = ep.tile([P, 1], F32, tag="rec")
                    nc.vector.reciprocal(rec, po[:, E:E + 1])
                    n_idx = (q0 + j * P) // P
                    nc.vector.tensor_scalar_mul(
                        out_sb[:, n_idx, :], po[:, 0:E], rec)

        nc.sync.dma_start(out=out.rearrange("(n p) e -> p n e", p=P),
                          in_=out_sb)


def build_nc(t1l=T1 // 2, t2=T2):
    nc = bacc.Bacc()
    q = nc.declare_dram_parameter("q", [t1l, D], F32, isOutput=False)
    k = nc.declare_dram_parameter("k", [t2, D], F32, isOutput=False)
    v = nc.declare_dram_parameter("v", [t2, D], F32, isOutput=False)
    mask = nc.declare_dram_parameter("mask", [t2], F32, isOutput=False)
    Wq = nc.declare_dram_parameter("Wq", [D, E], F32, isOutput=False)
    bq = nc.declare_dram_parameter("bq", [E], F32, isOutput=False)
    Wk = nc.declare_dram_parameter("Wk", [D, E], F32, isOutput=False)
    bk = nc.declare_dram_parameter("bk", [E], F32, isOutput=False)
    Wv = nc.declare_dram_parameter("Wv", [D, E], F32, isOutput=False)
    bv = nc.declare_dram_parameter("bv", [E], F32, isOutput=False)
    out = nc.declare_dram_parameter("out", [t1l, E], F32, isOutput=True)
    with tile.TileContext(nc) as tc:
        build_body(tc, nc, q[:], k[:], v[:], mask[:], Wq[:], bq[:], Wk[:],
                   bk[:], Wv[:], bv[:], out[:], t1l, t2)
    nc.compile()
    return nc


_NC_CACHE = {}


def _get_nc():
    if "nc" not in _NC_CACHE:
        _NC_CACHE["nc"] = build_nc()
    return _NC_CACHE["nc"]


def make_in_maps(q, k, v, mask, Wq, bq, Wk, bk, Wv, bv):
    t1l = T1 // 2
    shared = {
        "Wq": np.ascontiguousarray(Wq, np.float32),
        "bq": np.ascontiguousarray(bq, np.float32),
        "Wk": np.ascontiguousarray(Wk, np.float32),
        "bk": np.ascontiguousarray(bk, np.float32),
        "Wv": np.ascontiguousarray(Wv, np.float32),
        "bv": np.ascontiguousarray(bv, np.float32),
    }
    in_maps = []
    for c in range(8):
        b, h = divmod(c, 2)
        in_maps.append({
            "q": np.ascontiguousarray(q[b, h * t1l:(h + 1) * t1l], np.float32),
            "k": np.ascontiguousarray(k[b], np.float32),
            "v": np.ascontiguousarray(v[b], np.float32),
            "mask": np.ascontiguousarray(mask[b, 0], np.float32),
            **shared,
        })
    return in_maps


def assemble_out(results):
    t1l = T1 // 2
    out = np.empty((B, T1, E), np.float32)
    for c in range(8):
        b, h = divmod(c, 2)
        out[b, h * t1l:(h + 1) * t1l] = results[c]["out"]
    return out


def run(inputs, trace=False):
    from concourse.bass_utils import run_bass_kernel_spmd
    _install_ntff_hook()
    nc = _get_nc()
    in_maps = make_in_maps(**inputs)
    res = run_bass_kernel_spmd(nc, in_maps, list(range(8)), trace=trace)
    return assemble_out(res.results), res


def kernel(q, k, v, mask, Wq, bq, Wk, bk, Wv, bv):
    out, _ = run(dict(q=q, k=k, v=v, mask=mask, Wq=Wq, bq=bq, Wk=Wk, bk=bk,
                      Wv=Wv, bv=bv))
    return out
